# revision 12
# baseline (speedup 1.0000x reference)
"""Trainium2 Bass kernel for the depth-2 TT-compressed meta-linear module.

Math (per token t, with x the (D,)-vector of that token, repeated DEPTH=2):
    w0[r]      = sum_d x[d] * core0[0,d,r]
    y1[r,R]    = sum_d x[d] * core1[r,d,R]
    w1[R]      = sum_r w0[r] * y1[r,R]
    y2[r,R]    = sum_d x[d] * core2[r,d,R]
    w2[R]      = sum_r w1[r] * y2[r,R]
    x'[d]      = sum_R w2[R] * core3[R,d,0]
Output = x'' + bias.

Device mapping (8-way data parallel over tokens; 2048 tokens/core):
  - x tiles (128 tokens, D) are DMA'd in naturally, transposed on TensorE
    (128x128 tiles, batched 4-per-PSUM-bank) to get XT (d on partitions).
  - Depth 1: one 128-wide matmul pass computes [w0 replicated | y1]; a second
    64-wide pass computes y2; elementwise multiplies on VectorE fold w into y;
    a constant 0/1 matrix (SREP) does the r-sum on TensorE.
  - The depth boundary is linear, so depth 2's input contractions are folded
    on the host: M01 = C3S @ C01 and M2 = C3S @ C2 map z2 (depth-1 state)
    straight to depth-2's [w0|y1] and y2 — the intermediate x' is never
    materialized on device.
  - The final step uses an augmented w2 (with a ones row pairing a bias row
    in C3B) as the matmul *stationary* operand, producing output + bias
    directly in natural (token, d) layout.
  - float32r dtype is used for all matmul operands (full-rate fp32 path).
"""

import numpy as np

import concourse.bacc as bacc
import concourse.tile as tile
import concourse.mybir as mybir
import concourse.bass_utils as bass_utils

N_CORES = 8
B, N, D, R = 4, 4096, 1024, 8
T_TOTAL = B * N              # 16384 tokens
T_CORE = T_TOTAL // N_CORES  # 2048 tokens per core
TB = 512                     # tokens per pipeline block
NBLK = T_CORE // TB          # 4 blocks per core
NTILE = TB // 128            # 4 token-tiles per block
NCH = D // 128               # 8 d-chunks

F32R = mybir.dt.float32r
F32 = mybir.dt.float32


def _build_program(with_bias=False):
    nc = bacc.Bacc("TRN2", target_bir_lowering=False, debug=False,
                   num_devices=N_CORES)

    x_d = nc.dram_tensor("x", [T_CORE, D], F32R, kind="ExternalInput")
    out_d = nc.dram_tensor("out", [T_CORE, D], F32R, kind="ExternalOutput")
    c01_d = nc.dram_tensor("c01", [128, NCH * 128], F32R, kind="ExternalInput")
    c2_d = nc.dram_tensor("c2", [128, NCH * 64], F32R, kind="ExternalInput")
    srep_d = nc.dram_tensor("srep", [64, 64], F32R, kind="ExternalInput")
    s2_d = nc.dram_tensor("s2", [64, 8], F32R, kind="ExternalInput")
    m01_d = nc.dram_tensor("m01", [64, 128], F32R, kind="ExternalInput")
    m2_d = nc.dram_tensor("m2", [64, 64], F32R, kind="ExternalInput")
    c3b_d = nc.dram_tensor("c3b", [8, D], F32R, kind="ExternalInput")
    ident_d = nc.dram_tensor("ident", [128, 128], F32R, kind="ExternalInput")
    if with_bias:
        biasr_d = nc.dram_tensor("biasr", [128, D], F32R,
                                 kind="ExternalInput")

    x_ap = x_d.ap()
    out_ap = out_d.ap()

    with tile.TileContext(nc) as tc:
        with (
            tc.tile_pool(name="consts", bufs=1) as cpool,
            tc.tile_pool(name="xin", bufs=6) as pool_xin,
            tc.tile_pool(name="xt", bufs=2) as pool_xt,
            tc.tile_pool(name="z", bufs=3) as pool_z,
            tc.tile_pool(name="w2", bufs=2) as pool_w2,
            tc.tile_pool(name="outsb", bufs=3) as pool_out,
            tc.tile_pool(name="ps_t", bufs=2, space="PSUM") as ps_t,
            tc.tile_pool(name="ps_p1", bufs=2, space="PSUM") as ps_p1,
            tc.tile_pool(name="ps_p2", bufs=2, space="PSUM") as ps_p2,
            tc.tile_pool(name="ps_d", bufs=2, space="PSUM") as ps_d,
        ):
            # ---- constants into SBUF; ident first: transposes need only it,
            # so compute starts while the big constants stream in ----
            ident_s = cpool.tile([128, 128], F32R, tag="ident")
            nc.gpsimd.dma_start(ident_s[:], ident_d.ap()[:])
            c01_s = cpool.tile([128, NCH * 128], F32R, tag="c01")
            c2_s = cpool.tile([128, NCH * 64], F32R, tag="c2")
            srep_s = cpool.tile([64, 64], F32R, tag="srep")
            s2_s = cpool.tile([64, 8], F32R, tag="s2")
            m01_s = cpool.tile([64, 128], F32R, tag="m01")
            m2_s = cpool.tile([64, 64], F32R, tag="m2")
            c3b_s = cpool.tile([8, D], F32R, tag="c3b")
            if with_bias:
                biasr_s = cpool.tile([128, D], F32R, tag="biasr")

            def load_consts():
                nc.gpsimd.dma_start(c01_s[:], c01_d.ap()[:])
                nc.gpsimd.dma_start(c2_s[:], c2_d.ap()[:])
                nc.gpsimd.dma_start(srep_s[:], srep_d.ap()[:])
                nc.gpsimd.dma_start(s2_s[:], s2_d.ap()[:])
                nc.gpsimd.dma_start(m01_s[:], m01_d.ap()[:])
                nc.gpsimd.dma_start(m2_s[:], m2_d.ap()[:])
                nc.gpsimd.dma_start(c3b_s[:], c3b_d.ap()[:])
                if with_bias:
                    nc.gpsimd.dma_start(biasr_s[:], biasr_d.ap()[:])

            def z_chain(p1, p2):
                """z2 (64, TB) from the [w0|y1] and y2 PSUM passes.

                HW allows only one PSUM input per DVE op, so the w side is
                staged through SBUF on ScalarE."""
                w0s = pool_z.tile([64, TB], F32R, tag="w0s")
                nc.scalar.copy(w0s[:], p1[0:64, :])
                z1 = pool_z.tile([64, TB], F32R, tag="z1")
                nc.vector.tensor_mul(z1[:], p1[64:128, :], w0s[:])
                pw = ps_p1.tile([64, TB], F32, tag="p1")
                nc.tensor.matmul(pw[:], srep_s[:], z1[:], start=True, stop=True)
                pws = pool_z.tile([64, TB], F32R, tag="pws")
                nc.scalar.copy(pws[:], pw[:])
                z2 = pool_z.tile([64, TB], F32R, tag="z2")
                nc.vector.tensor_mul(z2[:], p2[:], pws[:])
                return z2

            for b in range(NBLK):
                # ---- load 4 token-tiles (natural layout) ----
                xins = []
                for i in range(NTILE):
                    xin = pool_xin.tile([128, D], F32R, tag="xin")
                    r0 = (b * NTILE + i) * 128
                    nc.sync.dma_start(xin[:], x_ap[r0:r0 + 128, :])
                    xins.append(xin)

                # ---- transpose to XT: 8 chunks of (128 d, TB tokens) ----
                xts = []
                for j in range(NCH):
                    ps = ps_t.tile([128, TB], F32R, tag="ps_t")
                    for i in range(NTILE):
                        nc.tensor.transpose(
                            ps[:, i * 128:(i + 1) * 128],
                            xins[i][:, j * 128:(j + 1) * 128],
                            ident_s[:])
                    xt_j = pool_xt.tile([128, TB], F32R, tag=f"xt{j}")
                    if j % 2 == 0:
                        nc.vector.tensor_copy(xt_j[:], ps[:])
                    else:
                        nc.scalar.copy(xt_j[:], ps[:])
                    xts.append(xt_j)
                    if b == 0 and j == 0:
                        load_consts()

                # ---- depth 1: [w0|y1] and y2 passes over XT ----
                p1 = ps_p1.tile([128, TB], F32, tag="p1")
                for j in range(NCH):
                    nc.tensor.matmul(p1[:], c01_s[:, j * 128:(j + 1) * 128],
                                     xts[j][:],
                                     start=(j == 0), stop=(j == NCH - 1))
                p2 = ps_p2.tile([64, TB], F32, tag="p2")
                for j in range(NCH):
                    nc.tensor.matmul(p2[:], c2_s[:, j * 64:(j + 1) * 64],
                                     xts[j][:],
                                     start=(j == 0), stop=(j == NCH - 1))
                z2d1 = z_chain(p1, p2)

                # ---- depth 2 via host-folded boundary: one matmul each ----
                p1b = ps_p1.tile([128, TB], F32, tag="p1")
                nc.tensor.matmul(p1b[:], m01_s[:], z2d1[:],
                                 start=True, stop=True)
                p2b = ps_p2.tile([64, TB], F32, tag="p2")
                nc.tensor.matmul(p2b[:], m2_s[:], z2d1[:],
                                 start=True, stop=True)
                z2d2 = z_chain(p1b, p2b)

                # ---- w2 (+ ones row for the bias) ----
                pw2 = ps_d.tile([8, TB], F32, tag="pd")
                nc.tensor.matmul(pw2[:], s2_s[:], z2d2[:],
                                 start=True, stop=True)
                w2s = pool_w2.tile([8, TB], F32R, tag="w2s")
                nc.scalar.copy(w2s[:, :], pw2[:])

                # ---- final: out[t, d] = w2aug^T @ C3B  (bias folded in) ----
                for i in range(NTILE):
                    out_sb = pool_out.tile([128, D], F32R, tag="outsb")
                    for h in range(2):
                        pf = ps_d.tile([128, 512], F32, tag="pd")
                        nc.tensor.matmul(pf[:],
                                         w2s[:, i * 128:(i + 1) * 128],
                                         c3b_s[:, h * 512:(h + 1) * 512],
                                         start=True, stop=True)
                        if with_bias:
                            nc.vector.tensor_add(
                                out_sb[:, h * 512:(h + 1) * 512], pf[:],
                                biasr_s[:, h * 512:(h + 1) * 512])
                        elif (i + h) % 2 == 0:
                            nc.vector.tensor_copy(
                                out_sb[:, h * 512:(h + 1) * 512], pf[:])
                        else:
                            nc.scalar.copy(
                                out_sb[:, h * 512:(h + 1) * 512], pf[:])
                    r0 = (b * NTILE + i) * 128
                    nc.sync.dma_start(out_ap[r0:r0 + 128, :], out_sb[:])

    nc.compile()
    return nc


def _constants(core0, core1, core2, core3, bias):
    core0 = np.asarray(core0, np.float32)
    core1 = np.asarray(core1, np.float32)
    core2 = np.asarray(core2, np.float32)
    core3 = np.asarray(core3, np.float32)
    bias = np.asarray(bias, np.float32)

    # k index = r*8 + R  (prev rank r, next rank R)
    C01 = np.zeros((D, 128), np.float32)
    C01[:, :64] = np.repeat(core0[0], R, axis=1)          # w0 replicated in R
    C01[:, 64:] = core1.transpose(1, 0, 2).reshape(D, 64)  # y1
    C2 = core2.transpose(1, 0, 2).reshape(D, 64)
    SREP = np.kron(np.ones((R, 1), np.float32),
                   np.kron(np.eye(R, dtype=np.float32),
                           np.ones((1, R), np.float32)))  # (64,64)
    S2 = np.tile(np.eye(R, dtype=np.float32), (R, 1))     # (64,8)
    C3S = np.tile(core3[:, :, 0], (R, 1))                 # (64,D)
    # host-folded depth boundary
    M01 = (C3S.astype(np.float64) @ C01.astype(np.float64)).astype(np.float32)
    M2 = (C3S.astype(np.float64) @ C2.astype(np.float64)).astype(np.float32)
    C3B = np.ascontiguousarray(core3[:, :, 0])            # (8,D)
    IDENT = np.eye(128, dtype=np.float32)

    def chunk_major(a, po):
        # (D, po) -> (128, NCH*po) with d-chunk along the free dim
        return np.ascontiguousarray(
            a.reshape(NCH, 128, po).transpose(1, 0, 2).reshape(128, NCH * po))

    consts = {
        "c01": chunk_major(C01, 128),
        "c2": chunk_major(C2, 64),
        "srep": np.ascontiguousarray(SREP),
        "s2": np.ascontiguousarray(S2),
        "m01": np.ascontiguousarray(M01),
        "m2": np.ascontiguousarray(M2),
        "c3b": C3B,
        "ident": IDENT,
    }
    if np.any(bias):
        consts["biasr"] = np.ascontiguousarray(
            np.tile(bias[None, :], (128, 1)))
    return consts


_NC_CACHE = {}


def _get_program(with_bias=False):
    if with_bias not in _NC_CACHE:
        _NC_CACHE[with_bias] = _build_program(with_bias)
    return _NC_CACHE[with_bias]


def run(x, core0, core1, core2, core3, bias, trace=False, **spmd_kwargs):
    consts = _constants(core0, core1, core2, core3, bias)
    nc = _get_program(with_bias="biasr" in consts)
    xf = np.ascontiguousarray(np.asarray(x, np.float32).reshape(T_TOTAL, D))
    in_maps = []
    for c in range(N_CORES):
        m = dict(consts)
        m["x"] = np.ascontiguousarray(xf[c * T_CORE:(c + 1) * T_CORE])
        in_maps.append(m)
    res = bass_utils.run_bass_kernel_spmd(
        nc, in_maps, core_ids=list(range(N_CORES)), trace=trace, **spmd_kwargs)
    out = np.concatenate([res.results[c]["out"] for c in range(N_CORES)],
                         axis=0)
    return out.reshape(B, N, D), res


def kernel(x, core0, core1, core2, core3, bias):
    out, _ = run(x, core0, core1, core2, core3, bias)
    return out
